# revision 25
# baseline (speedup 1.0000x reference)
"""AddContextFrames distributed Trainium2 kernel.

out[0, w*80+f, t] = signal[0, f, t + w - 9]  (zero outside), w in 0..18.

Strategy: shard the time axis across 8 NeuronCores. Each core receives a
zero-padded input shard (80, 4096+18) that already includes the halo, so no
inter-core communication is needed. On-core: one DMA load into SBUF, then 19
shifted-window DMA stores into the (1520, 4096) output shard.
"""

import numpy as np

import concourse.bass as bass
import concourse.mybir as mybir
from concourse.bass_utils import run_bass_kernel_spmd

N_CORES = 8
N_CONTEXT = 9
WINDOW = 2 * N_CONTEXT + 1  # 19
FEATS = 80
STEPS = 32768
SHARD = STEPS // N_CORES    # 4096
HALO = 2 * N_CONTEXT        # 18
IN_W = SHARD + HALO         # 4114
OUT_CH = WINDOW * FEATS     # 1520

_nc_cache = None


# Port-balanced SBUF layout with large DMA descriptors.  Each feature row is
# split into 2 time blocks of 2048 (sub-row s = 2f + b holds
# x[f, b*2048 : b*2048 + 2066], incl. 18-elem halo).  Region 0: sub-rows
# 0..127 on partition s.  Region 1: sub-rows 128..159 on partitions 4j
# (j = s - 128) — exactly 2 per SBUF AXI port, so all 16 ports carry equal
# load (10 sub-rows/port/window).  Stores issue in 1024-elem halves → 4 KB
# descriptors, few enough (~6400 total) that single-ring HWDGE descriptor
# generation (~5.3 ns/desc) stays well under the ~68 us HBM time.
TB = 2048            # time block
HB = TB // 2         # 1024 store/load half
SUBW = TB + HALO     # 2066
SUBH = HB + HALO     # 1042 (half load width)
PITCH = 2072         # sub-row pitch in elements (32B aligned)


import os

VARIANT = os.environ.get("KVAR", "v5")


def build_nc(variant: str = None) -> bass.Bass:
    from concourse.ap import AP

    if variant is None:
        variant = VARIANT

    nc = bass.Bass()
    x = nc.declare_dram_parameter(
        "signal", [FEATS, IN_W], mybir.dt.float32, isOutput=False
    )
    out = nc.declare_dram_parameter(
        "out", [OUT_CH, SHARD], mybir.dt.float32, isOutput=True
    )
    from contextlib import ExitStack

    with ExitStack() as stack:
        nslot = 3 if variant in ("v10", "v11") else 2
        tile = stack.enter_context(
            nc.sbuf_tensor([128, nslot, PITCH], mybir.dt.float32)
        )

        ld = [
            [stack.enter_context(nc.semaphore(f"ld{r}{h}")) for h in (0, 1)]
            for r in (0, 1)
        ]
        ss = stack.enter_context(nc.semaphore("ss"))
        lt = [stack.enter_context(nc.semaphore(f"lt{i}")) for i in range(3)]
        block = stack.enter_context(nc.Block())

        @block.sync
        def _(sync):
            # loads: disjoint sub-row column halves [0:1042) and [1042:2066).
            # h=1 stores read [w+1024, w+2048) which spans both halves; the
            # issue order (h0 stores, with their waits, precede h1) covers it.
            # Region-1 load first (smallest -> earliest store release);
            # region-0 loads split into partition halves so both run in
            # parallel on separate queues.
            if variant == "v14":
                # v9 with loads and stores split into column halves: the
                # first store releases after only half the region-0 load.
                FS = FEATS * SHARD
                PP = 2 * PITCH
                th = tile.tensor if hasattr(tile, "tensor") else tile
                W = WINDOW
                for h, c0, cw in ((0, 0, SUBH), (1, SUBH, HB)):
                    sync.dma_start(
                        out=tile[0:128:4, 1, c0 : c0 + cw],
                        in_=AP(x, 64 * IN_W + c0, [[IN_W, 16], [TB, 2], [1, cw]]),
                    ).then_inc(ld[1][h], 16)
                    sync.dma_start(
                        out=tile[:, 0, c0 : c0 + cw],
                        in_=AP(x, c0, [[IN_W, 64], [TB, 2], [1, cw]]),
                    ).then_inc(ld[0][h], 16)
                # h=0 stores cover cols w+[0:HB) (within load half 0);
                # h=1 stores cover w+[HB:TB) (needs both halves, but the
                # preceding h=0 waits already cover half 0).
                n = 0
                for h in (0, 1):
                    for r in (1, 0):
                        sync.wait_ge(ld[r][h], 16)
                        if r == 1:
                            sync.dma_start(
                                out=AP(out, 128 * TB + h * HB,
                                       [[TB, 32], [FS, W], [1, HB]]),
                                in_=AP(th, PITCH + h * HB,
                                       [[4 * PP, 32], [1, W], [1, HB]]),
                            ).then_inc(ss, 16)
                        else:
                            sync.dma_start(
                                out=AP(out, h * HB,
                                       [[TB, 128], [FS, W], [1, HB]]),
                                in_=AP(th, h * HB,
                                       [[PP, 128], [1, W], [1, HB]]),
                            ).then_inc(ss, 16)
                        n += 1
                sync.wait_ge(ss, 16 * n)
                return

            if variant == "v12":
                # v9 with the region-0 load split into partition halves so
                # both halves stream on separate queues and the big store
                # can start ~2us sooner.
                FS = FEATS * SHARD
                PP = 2 * PITCH
                th = tile.tensor if hasattr(tile, "tensor") else tile
                W = WINDOW
                sync.dma_start(
                    out=tile[0:128:4, 1, 0:SUBW],
                    in_=AP(x, 64 * IN_W, [[IN_W, 16], [TB, 2], [1, SUBW]]),
                ).then_inc(ld[1][0], 16)
                sync.dma_start(
                    out=tile[0:64, 0, 0:SUBW],
                    in_=AP(x, 0, [[IN_W, 32], [TB, 2], [1, SUBW]]),
                ).then_inc(ld[0][0], 16)
                sync.dma_start(
                    out=tile[64:128, 0, 0:SUBW],
                    in_=AP(x, 32 * IN_W, [[IN_W, 32], [TB, 2], [1, SUBW]]),
                ).then_inc(ld[0][0], 16)
                sync.wait_ge(ld[1][0], 16)
                sync.dma_start(
                    out=AP(out, 128 * TB, [[TB, 32], [FS, W], [1, TB]]),
                    in_=AP(th, PITCH, [[4 * PP, 32], [1, W], [1, TB]]),
                ).then_inc(ss, 16)
                sync.wait_ge(ld[0][0], 32)
                sync.dma_start(
                    out=AP(out, 0, [[TB, 128], [FS, W], [1, TB]]),
                    in_=AP(th, 0, [[PP, 128], [1, W], [1, TB]]),
                ).then_inc(ss, 16)
                sync.wait_ge(ss, 32)
                return

            if variant == "v11":
                # v9 + port-15 offload (legal-AP form).  SDMA engine/port 15
                # (SBUF partitions 92-95, 124-127) runs ~1.19x slow in ~half
                # of runs, stretching the whole kernel.  Shorten the stored
                # width of its 8 region-0 sub-rows to TB-C; the C-column
                # tails are served from single-partition replicas on donors
                # {2,6,..,30} (slot 2) — one per even port.  Port 15 then
                # carries 10-8*C/TB row-units vs 10+C/TB elsewhere, which
                # equalizes under the 1.19x slowdown and costs only ~1.8%
                # when port 15 runs at full speed.
                C = 368
                CT = C + HALO
                M = TB - C
                FS = FEATS * SHARD
                PP = 3 * PITCH
                S2 = 2 * PITCH
                th = tile.tensor if hasattr(tile, "tensor") else tile
                W = WINDOW
                # main loads
                sync.dma_start(
                    out=tile[0:128:4, 1, 0:SUBW],
                    in_=AP(x, 64 * IN_W, [[IN_W, 16], [TB, 2], [1, SUBW]]),
                ).then_inc(ld[1][0], 16)
                sync.dma_start(
                    out=tile[:, 0, 0:SUBW],
                    in_=AP(x, 0, [[IN_W, 64], [TB, 2], [1, SUBW]]),
                ).then_inc(ld[0][0], 16)
                # tail replicas: row s (s=92..95, 124..127) -> donor
                # partition d, slot 2: x[s//2, (s%2)*TB + M : +CT]
                donors = list(zip((92, 93, 94, 95, 124, 125, 126, 127),
                                  (2, 6, 10, 14, 18, 22, 26, 30)))
                for s_row, d in donors:
                    sync.dma_start(
                        out=tile[d : d + 1, 2, 0:CT],
                        in_=AP(
                            x,
                            (s_row // 2) * IN_W + (s_row % 2) * TB + M,
                            [[1, CT]],
                        ),
                    ).then_inc(lt[0], 16)
                # region-1 store (stride-4 base-0, untouched)
                sync.wait_ge(ld[1][0], 16)
                sync.dma_start(
                    out=AP(out, 128 * TB, [[TB, 32], [FS, W], [1, TB]]),
                    in_=AP(th, PITCH, [[4 * PP, 32], [1, W], [1, TB]]),
                ).then_inc(ss, 16)
                # tail stores from donors
                sync.wait_ge(lt[0], 16 * len(donors))
                for s_row, d in donors:
                    sync.dma_start(
                        out=AP(out, s_row * TB + M, [[FS, W], [1, C]]),
                        in_=AP(th, d * PP + S2, [[PP, 1], [1, W], [1, C]]),
                    ).then_inc(ss, 16)
                # region-0 stores: full [0:92) and [96:124), short 92-95
                # and 124-127
                sync.wait_ge(ld[0][0], 16)
                sync.dma_start(
                    out=AP(out, 0, [[TB, 92], [FS, W], [1, TB]]),
                    in_=AP(th, 0, [[PP, 92], [1, W], [1, TB]]),
                ).then_inc(ss, 16)
                sync.dma_start(
                    out=AP(out, 96 * TB, [[TB, 28], [FS, W], [1, TB]]),
                    in_=AP(th, 96 * PP, [[PP, 28], [1, W], [1, TB]]),
                ).then_inc(ss, 16)
                sync.dma_start(
                    out=AP(out, 92 * TB, [[TB, 4], [FS, W], [1, M]]),
                    in_=AP(th, 92 * PP, [[PP, 4], [1, W], [1, M]]),
                ).then_inc(ss, 16)
                sync.dma_start(
                    out=AP(out, 124 * TB, [[TB, 4], [FS, W], [1, M]]),
                    in_=AP(th, 124 * PP, [[PP, 4], [1, W], [1, M]]),
                ).then_inc(ss, 16)
                sync.wait_ge(ss, 16 * 13)
                return

            if variant == "v10":
                # v9 + port-15 column offload.  SDMA engine/port 15 (SBUF
                # partitions 92-95, 124-127) runs ~1.19x slow in ~half of
                # runs.  Shorten those partitions' stored width to TB-C and
                # serve the C-column tails from replicas on donor partitions
                # (slot-2 space, one per even port: {2,6,...,30}; odd-port
                # donors {66,70} for the two region-1 rows).  Port 15 then
                # carries 10*(1-C/TB) row-units vs 10+C/TB for the rest --
                # balanced exactly when port 15 runs 1.19x slow, and only
                # ~1.5% worse than ideal when it doesn't.
                C = 304
                CT = C + HALO
                M = TB - C
                FS = FEATS * SHARD
                PP = 3 * PITCH
                S2 = 2 * PITCH
                th = tile.tensor if hasattr(tile, "tensor") else tile
                W = WINDOW
                # tail loads into slot 2, disjoint col ranges per group:
                # s=92..95 (f=46,47) -> {2,6,10,14} @ [0:CT);
                # s=124..127 (f=62,63) -> {18,22,26,30} @ [CT:2CT);
                # s=151,159 (f=75,79, b=1) -> {66,70} @ [2CT:3CT)
                sync.dma_start(
                    out=tile[2:15:4, 2, 0:CT],
                    in_=AP(x, 46 * IN_W + M, [[IN_W, 2], [TB, 2], [1, CT]]),
                ).then_inc(lt[0], 16)
                sync.dma_start(
                    out=tile[18:31:4, 2, CT : 2 * CT],
                    in_=AP(x, 62 * IN_W + M, [[IN_W, 2], [TB, 2], [1, CT]]),
                ).then_inc(lt[1], 16)
                sync.dma_start(
                    out=tile[66:71:4, 2, 2 * CT : 3 * CT],
                    in_=AP(x, 75 * IN_W + TB + M, [[4 * IN_W, 2], [1, CT]]),
                ).then_inc(lt[2], 16)
                # main loads
                sync.dma_start(
                    out=tile[0:128:4, 1, 0:SUBW],
                    in_=AP(x, 64 * IN_W, [[IN_W, 16], [TB, 2], [1, SUBW]]),
                ).then_inc(ld[1][0], 16)
                sync.dma_start(
                    out=tile[:, 0, 0:SUBW],
                    in_=AP(x, 0, [[IN_W, 64], [TB, 2], [1, SUBW]]),
                ).then_inc(ld[0][0], 16)
                # region-1 stores: full rows j in [0,23) and [24,31),
                # short rows j=23,31 (partitions 92,124)
                sync.wait_ge(ld[1][0], 16)
                sync.dma_start(
                    out=AP(out, 128 * TB, [[TB, 23], [FS, W], [1, TB]]),
                    in_=AP(th, PITCH, [[4 * PP, 23], [1, W], [1, TB]]),
                ).then_inc(ss, 16)
                sync.dma_start(
                    out=AP(out, 152 * TB, [[TB, 7], [FS, W], [1, TB]]),
                    in_=AP(th, 96 * PP + PITCH, [[4 * PP, 7], [1, W], [1, TB]]),
                ).then_inc(ss, 16)
                sync.dma_start(
                    out=AP(out, 151 * TB, [[8 * TB, 2], [FS, W], [1, M]]),
                    in_=AP(th, 92 * PP + PITCH, [[32 * PP, 2], [1, W], [1, M]]),
                ).then_inc(ss, 16)
                # tail stores from donors
                for s_ in lt:
                    sync.wait_ge(s_, 16)
                sync.dma_start(
                    out=AP(out, 92 * TB + M, [[TB, 4], [FS, W], [1, C]]),
                    in_=AP(th, 2 * PP + S2, [[4 * PP, 4], [1, W], [1, C]]),
                ).then_inc(ss, 16)
                sync.dma_start(
                    out=AP(out, 124 * TB + M, [[TB, 4], [FS, W], [1, C]]),
                    in_=AP(th, 18 * PP + S2 + CT, [[4 * PP, 4], [1, W], [1, C]]),
                ).then_inc(ss, 16)
                sync.dma_start(
                    out=AP(out, 151 * TB + M, [[8 * TB, 2], [FS, W], [1, C]]),
                    in_=AP(th, 66 * PP + S2 + 2 * CT, [[4 * PP, 2], [1, W], [1, C]]),
                ).then_inc(ss, 16)
                # region-0 stores: full partitions [0:92) and [96:124),
                # short partitions 92-95 and 124-127
                sync.wait_ge(ld[0][0], 16)
                sync.dma_start(
                    out=AP(out, 0, [[TB, 92], [FS, W], [1, TB]]),
                    in_=AP(th, 0, [[PP, 92], [1, W], [1, TB]]),
                ).then_inc(ss, 16)
                sync.dma_start(
                    out=AP(out, 96 * TB, [[TB, 28], [FS, W], [1, TB]]),
                    in_=AP(th, 96 * PP, [[PP, 28], [1, W], [1, TB]]),
                ).then_inc(ss, 16)
                sync.dma_start(
                    out=AP(out, 92 * TB, [[TB, 4], [FS, W], [1, M]]),
                    in_=AP(th, 92 * PP, [[PP, 4], [1, W], [1, M]]),
                ).then_inc(ss, 16)
                sync.dma_start(
                    out=AP(out, 124 * TB, [[TB, 4], [FS, W], [1, M]]),
                    in_=AP(th, 124 * PP, [[PP, 4], [1, W], [1, M]]),
                ).then_inc(ss, 16)
                sync.wait_ge(ss, 160)
                return

            if variant == "v9":
                # Whole-kernel minimal-DMA form: 2 loads + 2 stores.  The
                # SBUF side of each store is [[pitch,128],[1,19],[1,2048]]
                # (partition, window, time) whose element order matches the
                # fully-contiguous DRAM output — all 19 windows of a region
                # in ONE DMA.  Per-DMA fixed costs vanish; every DMA spans
                # partitions covering all 16 SBUF ports uniformly.
                FS = FEATS * SHARD
                PP = 2 * PITCH  # flat elements per partition
                th = tile.tensor if hasattr(tile, "tensor") else tile
                sync.dma_start(
                    out=tile[0:128:4, 1, 0:SUBW],
                    in_=AP(x, 64 * IN_W, [[IN_W, 16], [TB, 2], [1, SUBW]]),
                ).then_inc(ld[1][0], 16)
                sync.dma_start(
                    out=tile[:, 0, 0:SUBW],
                    in_=AP(x, 0, [[IN_W, 64], [TB, 2], [1, SUBW]]),
                ).then_inc(ld[0][0], 16)
                sync.wait_ge(ld[1][0], 16)
                sync.dma_start(
                    out=AP(
                        out,
                        128 * TB,
                        [[TB, 32], [FS, WINDOW], [1, TB]],
                    ),
                    in_=AP(th, PITCH, [[4 * PP, 32], [1, WINDOW], [1, TB]]),
                ).then_inc(ss, 16)
                sync.wait_ge(ld[0][0], 16)
                sync.dma_start(
                    out=AP(out, 0, [[TB, 128], [FS, WINDOW], [1, TB]]),
                    in_=AP(th, 0, [[PP, 128], [1, WINDOW], [1, TB]]),
                ).then_inc(ss, 16)
                sync.wait_ge(ss, 32)
                return

            if variant == "v8":
                # v6 + engine-15 load shaping.  SDMA engine 15 runs ~1.19x
                # slower than peers in ~75% of runs (whole-kernel straggler).
                # Descriptors are dealt to engines by index, so a 15-desc DMA
                # gives engine 15 nothing (true for round-robin and blocked
                # dealing alike).  Per window: 2x15-row + 6x16-row DMAs for
                # region 0 (+ a 2-row eighth-width remainder that spreads
                # over all 16), 2x16-row DMAs for region 1.  Engines 0-14
                # carry 10.125 row-units, engine 15 carries 8.125 — its
                # ~1.19x slowdown then never sets the makespan.
                FS = FEATS * SHARD
                sync.dma_start(
                    out=tile[0:128:4, 1, 0:SUBW],
                    in_=AP(x, 64 * IN_W, [[IN_W, 16], [TB, 2], [1, SUBW]]),
                ).then_inc(ld[1][0], 16)
                sync.dma_start(
                    out=tile[:, 0, 0:SUBW],
                    in_=AP(x, 0, [[IN_W, 64], [TB, 2], [1, SUBW]]),
                ).then_inc(ld[0][0], 16)
                n = 0
                sync.wait_ge(ld[1][0], 16)
                for w in range(WINDOW):
                    # region 1: rows j in [0:16) and [16:32)
                    sync.dma_start(
                        out=AP(out, w * FS + 128 * TB, [[TB, 16], [1, TB]]),
                        in_=tile[0:61:4, 1, w : w + TB],
                    ).then_inc(ss, 16)
                    sync.dma_start(
                        out=AP(out, w * FS + 144 * TB, [[TB, 16], [1, TB]]),
                        in_=tile[64:125:4, 1, w : w + TB],
                    ).then_inc(ss, 16)
                    n += 2
                sync.wait_ge(ld[0][0], 16)
                for w in range(WINDOW):
                    # region 0: rows [0:15), [15:30) (15-desc: engine 15
                    # idle), [30:126) as 6x16, [126:128) as eighth-width
                    for lo, cnt in ((0, 15), (15, 15)) + tuple(
                        (30 + 16 * i, 16) for i in range(6)
                    ):
                        sync.dma_start(
                            out=AP(
                                out, w * FS + lo * TB, [[TB, cnt], [1, TB]]
                            ),
                            in_=tile[lo : lo + cnt, 0, w : w + TB],
                        ).then_inc(ss, 16)
                        n += 1
                    sync.dma_start(
                        out=AP(
                            out,
                            w * FS + 126 * TB,
                            [[TB, 2], [256, 8], [1, 256]],
                        ),
                        in_=tile[126:128, 0, w : w + TB],
                    ).then_inc(ss, 16)
                    n += 1
                sync.wait_ge(ss, 16 * n)
                return

            if variant == "v7":
                # v6 + engine-15 offload: SDMA engine 15 (serving SBUF ports
                # of partitions 92-95 and 124-127) runs ~1.19x slower than
                # its peers in ~75% of runs, making it the makespan tail.
                # Offload the last C columns of each sub-row resident on
                # those partitions to replica buffers on 10 donor partitions
                # (2 mod 4 — slot 1 there is unused) spread over engines 0-9.
                C = 320      # offloaded tail columns (~15.6% of TB)
                CT = C + HALO
                M = TB - C   # shortened main width
                FS = FEATS * SHARD
                # tail loads: rows s=92..95 (f=46,47 b=0,1), s=124..127
                # (f=62,63), s=151 (f=75 b=1), s=159 (f=79 b=1)
                sync.dma_start(
                    out=tile[2:18:4, 1, 0:CT],
                    in_=AP(x, 46 * IN_W + M, [[IN_W, 2], [TB, 2], [1, CT]]),
                ).then_inc(ld[1][1], 16)
                sync.dma_start(
                    out=tile[66:82:4, 1, 0:CT],
                    in_=AP(x, 62 * IN_W + M, [[IN_W, 2], [TB, 2], [1, CT]]),
                ).then_inc(ld[1][1], 16)
                sync.dma_start(
                    out=tile[18:19, 1, 0:CT],
                    in_=AP(x, 75 * IN_W + TB + M, [[1, CT]]),
                ).then_inc(ld[1][1], 16)
                sync.dma_start(
                    out=tile[82:83, 1, 0:CT],
                    in_=AP(x, 79 * IN_W + TB + M, [[1, CT]]),
                ).then_inc(ld[1][1], 16)
                # main loads (as v6)
                sync.dma_start(
                    out=tile[0:128:4, 1, 0:SUBW],
                    in_=AP(x, 64 * IN_W, [[IN_W, 16], [TB, 2], [1, SUBW]]),
                ).then_inc(ld[1][0], 16)
                sync.dma_start(
                    out=tile[:, 0, 0:SUBW],
                    in_=AP(x, 0, [[IN_W, 64], [TB, 2], [1, SUBW]]),
                ).then_inc(ld[0][0], 16)
                n = 0
                sync.wait_ge(ld[1][1], 64)
                sync.wait_ge(ld[1][0], 16)
                for w in range(WINDOW):
                    # region-1 full rows j in [0,23) and [24,31)
                    sync.dma_start(
                        out=AP(out, w * FS + 128 * TB, [[TB, 23], [1, TB]]),
                        in_=tile[0:89:4, 1, w : w + TB],
                    ).then_inc(ss, 16)
                    sync.dma_start(
                        out=AP(out, w * FS + 152 * TB, [[TB, 7], [1, TB]]),
                        in_=tile[96:121:4, 1, w : w + TB],
                    ).then_inc(ss, 16)
                    # region-1 shortened rows j=23 (s=151), j=31 (s=159)
                    sync.dma_start(
                        out=AP(out, w * FS + 151 * TB, [[1, M]]),
                        in_=tile[92:93, 1, w : w + M],
                    ).then_inc(ss, 16)
                    sync.dma_start(
                        out=AP(out, w * FS + 159 * TB, [[1, M]]),
                        in_=tile[124:125, 1, w : w + M],
                    ).then_inc(ss, 16)
                    # tails of s=151, s=159 from donors 18, 82
                    sync.dma_start(
                        out=AP(out, w * FS + 151 * TB + M, [[1, C]]),
                        in_=tile[18:19, 1, w : w + C],
                    ).then_inc(ss, 16)
                    sync.dma_start(
                        out=AP(out, w * FS + 159 * TB + M, [[1, C]]),
                        in_=tile[82:83, 1, w : w + C],
                    ).then_inc(ss, 16)
                    n += 6
                sync.wait_ge(ld[0][0], 16)
                for w in range(WINDOW):
                    # region-0: full partitions [0:92) and [96:124)
                    sync.dma_start(
                        out=AP(out, w * FS, [[TB, 92], [1, TB]]),
                        in_=tile[0:92, 0, w : w + TB],
                    ).then_inc(ss, 16)
                    sync.dma_start(
                        out=AP(out, w * FS + 96 * TB, [[TB, 28], [1, TB]]),
                        in_=tile[96:124, 0, w : w + TB],
                    ).then_inc(ss, 16)
                    # shortened engine-15 partitions
                    sync.dma_start(
                        out=AP(out, w * FS + 92 * TB, [[TB, 4], [1, M]]),
                        in_=tile[92:96, 0, w : w + M],
                    ).then_inc(ss, 16)
                    sync.dma_start(
                        out=AP(out, w * FS + 124 * TB, [[TB, 4], [1, M]]),
                        in_=tile[124:128, 0, w : w + M],
                    ).then_inc(ss, 16)
                    # tails from donors {2,6,10,14} and {66,70,74,78}
                    sync.dma_start(
                        out=AP(out, w * FS + 92 * TB + M, [[TB, 4], [1, C]]),
                        in_=tile[2:18:4, 1, w : w + C],
                    ).then_inc(ss, 16)
                    sync.dma_start(
                        out=AP(out, w * FS + 124 * TB + M, [[TB, 4], [1, C]]),
                        in_=tile[66:82:4, 1, w : w + C],
                    ).then_inc(ss, 16)
                    n += 6
                sync.wait_ge(ss, 16 * n)
                return

            if variant == "v6":
                # Full-width loads and stores: 2 loads + 38 stores, every
                # store a fully-contiguous 1 MB / 256 KB DRAM write.
                sync.dma_start(
                    out=tile[0:128:4, 1, 0:SUBW],
                    in_=AP(x, 64 * IN_W, [[IN_W, 16], [TB, 2], [1, SUBW]]),
                ).then_inc(ld[1][0], 16)
                sync.dma_start(
                    out=tile[:, 0, 0:SUBW],
                    in_=AP(x, 0, [[IN_W, 64], [TB, 2], [1, SUBW]]),
                ).then_inc(ld[0][0], 16)
                n = 0
                for r in (1, 0):
                    sync.wait_ge(ld[r][0], 16)
                    for w in range(WINDOW):
                        if r == 0:
                            sb = tile[:, 0, w : w + TB]
                            dst = AP(
                                out, w * FEATS * SHARD, [[TB, 128], [1, TB]]
                            )
                        else:
                            sb = tile[0:128:4, 1, w : w + TB]
                            dst = AP(
                                out,
                                w * FEATS * SHARD + 128 * TB,
                                [[TB, 32], [1, TB]],
                            )
                        sync.dma_start(out=dst, in_=sb).then_inc(ss, 16)
                        n += 1
                sync.wait_ge(ss, 16 * n)
                return

            split_r0 = variant == "v5"
            for h, c0, cw in ((0, 0, SUBH), (1, SUBH, HB)):
                sync.dma_start(
                    out=tile[0:128:4, 1, c0 : c0 + cw],
                    in_=AP(
                        x, 64 * IN_W + c0, [[IN_W, 16], [TB, 2], [1, cw]]
                    ),
                ).then_inc(ld[1][h], 16)
                if split_r0:
                    sync.dma_start(
                        out=tile[0:64, 0, c0 : c0 + cw],
                        in_=AP(x, c0, [[IN_W, 32], [TB, 2], [1, cw]]),
                    ).then_inc(ld[0][h], 16)
                    sync.dma_start(
                        out=tile[64:128, 0, c0 : c0 + cw],
                        in_=AP(
                            x, 32 * IN_W + c0, [[IN_W, 32], [TB, 2], [1, cw]]
                        ),
                    ).then_inc(ld[0][h], 16)
                else:
                    sync.dma_start(
                        out=tile[:, 0, c0 : c0 + cw],
                        in_=AP(x, c0, [[IN_W, 64], [TB, 2], [1, cw]]),
                    ).then_inc(ld[0][h], 16)
            n = 0
            # Region-1 stores first in each half: 19 small DMAs seed all the
            # DMA queues while the big region-0 load is still in flight.
            r_order = (1, 0) if variant == "v5" else (0, 1)
            for h in (0, 1):
                for r in r_order:
                    sync.wait_ge(
                        ld[r][h], 32 if (r == 0 and split_r0) else 16
                    )
                    for w in range(WINDOW):
                        if r == 0:
                            sb = tile[:, 0, w + h * HB : w + h * HB + HB]
                            dst = AP(
                                out,
                                w * FEATS * SHARD + h * HB,
                                [[TB, 128], [1, HB]],
                            )
                        else:
                            sb = tile[0:128:4, 1, w + h * HB : w + h * HB + HB]
                            dst = AP(
                                out,
                                w * FEATS * SHARD + 128 * TB + h * HB,
                                [[TB, 32], [1, HB]],
                            )
                        sync.dma_start(out=dst, in_=sb).then_inc(ss, 16)
                        n += 1
            sync.wait_ge(ss, 16 * n)

    return nc


def _install_ntff_hook():
    """The image lacks antenv.axon_hooks; synthesize it so trace=True works."""
    import sys, types

    if "antenv.axon_hooks" in sys.modules:
        return
    try:
        from trn_agent_boot.trn_boot import _ntff_profile_via_ctypes

        mod = types.ModuleType("antenv.axon_hooks")
        _state = {"hook": _ntff_profile_via_ctypes("/opt/axon/libaxon_pjrt.so")}
        mod.get_axon_ntff_profile_hook = lambda: _state["hook"]
        mod.set_axon_ntff_profile_hook = lambda h: _state.__setitem__("hook", h)
        sys.modules["antenv.axon_hooks"] = mod
        import antenv

        antenv.axon_hooks = mod
    except Exception:
        pass


def run(signal: np.ndarray, trace: bool = False):
    """signal: (1, 80, 32768) f32 -> ((1, 1520, 32768) f32, exec_time_ns|None)"""
    global _nc_cache
    if trace:
        _install_ntff_hook()
    signal = np.asarray(signal, dtype=np.float32)
    xp = np.zeros((FEATS, STEPS + HALO), np.float32)
    xp[:, N_CONTEXT : N_CONTEXT + STEPS] = signal[0]
    in_maps = [
        {"signal": np.ascontiguousarray(xp[:, i * SHARD : i * SHARD + IN_W])}
        for i in range(N_CORES)
    ]
    if _nc_cache is None:
        _nc_cache = build_nc()
    res = run_bass_kernel_spmd(
        _nc_cache, in_maps, core_ids=list(range(N_CORES)), trace=trace
    )
    out = np.empty((1, OUT_CH, STEPS), np.float32)
    for i in range(N_CORES):
        out[0, :, i * SHARD : (i + 1) * SHARD] = np.asarray(res.results[i]["out"])
    return out, res


def kernel(signal: np.ndarray) -> np.ndarray:
    out, _ = run(signal, trace=False)
    return out
